# revision 28
# baseline (speedup 1.0000x reference)
"""MiniTransformer block on 8 Trainium2 NeuronCores.

Sharding: pure data-parallel over batch (B=8 -> 1 batch element per core,
no collectives). Per core the full transformer block (LN -> single-head
attention -> residual -> LN -> MLP -> residual) runs as one Bass/Tile kernel.

Key design points:
  * All matmuls run in float32r (TF32-like, 1 cycle/row on the PE at free
    dim >= 256 vs 4 cycles/row for fp32; measured fro rel err ~1.5e-4).
  * Activations for matmul consumption are kept transposed ([feature, token])
    so projections chain without transposes; only LN outputs are transposed
    (PE transpose, 4 per 128-row chunk).
  * Attention scores are computed via a host-folded Wu = Wk @ Wq^T:
    scores^T = (h Wu) . h, so only ONE projection (u) is materialized
    instead of q and k.
  * Softmax: scores are computed transposed [t, s]; exp (with the 1/sqrt(D)
    scale fused) happens on the ScalarE during PSUM eviction; no max
    subtraction (LN-bounded scores, fp32 exp range is ample); the
    denominator comes from an extra ones-column appended to v, landing in
    PSUM as a per-partition scalar; normalization + residual add fold into
    a single scalar_tensor_tensor eviction.
  * (p @ v) @ Wo is computed as p @ (v (Wv Wo)) via host-folded Wvo,
    removing a projection and a transpose.
  * DMA traffic is spread across the SP/ACT HWDGE queues and the Pool SWDGE
    queue so no engine's sequencer stalls compute.
"""

import numpy as np

S, D, F, P = 2048, 512, 2048, 128
SC, DC, FC = S // P, D // P, F // P  # 16, 4, 16
SB = 256                             # attention s-block
NB = S // SB                         # 8
CPB = SB // P                        # s-chunks per attention block = 2
SBM = 512                            # MLP s-block
NBM = S // SBM                       # 4
CPBM = SBM // P                      # s-chunks per MLP block = 4
NCORES = 8
LN_EPS = 1e-5
ATTN_SCALE = float(1.0 / np.sqrt(np.float32(D)))

_CACHE = {}


def _build(has_affine1, has_affine2):
    import concourse.bass as bass
    import concourse.mybir as mybir
    import concourse.tile as tile
    from concourse import bacc
    from concourse.masks import make_identity
    from contextlib import ExitStack

    f32 = mybir.dt.float32
    f32r = mybir.dt.float32r
    AF = mybir.ActivationFunctionType
    OP = mybir.AluOpType

    nc = bacc.Bacc("TRN2", target_bir_lowering=False, debug=False,
                   num_devices=NCORES)

    x_d = nc.dram_tensor("x", [S, D], f32, kind="ExternalInput").ap()
    wu_d = nc.dram_tensor("wu", [D, D], f32r, kind="ExternalInput").ap()
    wvo_d = nc.dram_tensor("wvo", [D, D], f32r, kind="ExternalInput").ap()
    w1_d = nc.dram_tensor("w1", [D, F], f32r, kind="ExternalInput").ap()
    w2_d = nc.dram_tensor("w2", [F, D], f32r, kind="ExternalInput").ap()
    bf_d = nc.dram_tensor("bf", [F], f32, kind="ExternalInput").ap()
    b2_d = nc.dram_tensor("b2", [D], f32, kind="ExternalInput").ap()
    g1_d = nc.dram_tensor("g1", [D], f32, kind="ExternalInput").ap()
    be1_d = nc.dram_tensor("be1", [D], f32, kind="ExternalInput").ap()
    g2_d = nc.dram_tensor("g2", [D], f32, kind="ExternalInput").ap()
    be2_d = nc.dram_tensor("be2", [D], f32, kind="ExternalInput").ap()
    out_d = nc.dram_tensor("out", [S, D], f32, kind="ExternalOutput").ap()

    x_r = x_d.rearrange("(sc p) d -> p sc d", p=P)      # [128, 16, 512]
    out_r = out_d.rearrange("(sc p) d -> p sc d", p=P)
    wu_r = wu_d.rearrange("(ko ki) n -> ki ko n", ki=P)  # [128, 4, 512]
    wvo_r = wvo_d.rearrange("(ko ki) n -> ki ko n", ki=P)
    w1_r = w1_d.rearrange("(ko ki) n -> ki ko n", ki=P)  # [128, 4, 2048]
    w2_r = w2_d.rearrange("(ko ki) n -> ki ko n", ki=P)  # [128, 16, 512]
    bf_r = bf_d.rearrange("(o p) -> p o", p=P)           # [128, 16]

    def bcast(ap):  # [D] -> [128, D] partition-broadcast (for DMA only)
        return bass.AP(tensor=ap.tensor, offset=ap.offset, ap=[[0, P], [1, D]])

    with tile.TileContext(nc) as tc, ExitStack() as top:
        long_pool = top.enter_context(tc.tile_pool(name="long", bufs=1))
        const_pool = top.enter_context(tc.tile_pool(name="consts", bufs=1))
        w1_pool = top.enter_context(tc.tile_pool(name="w1p", bufs=1))
        tmpBC = top.enter_context(tc.tile_pool(name="tmpBC", bufs=2))

        # ---- constants / small tiles -------------------------------------
        ident = const_pool.tile([P, P], f32r)
        with tc.tile_pool(name="identf", bufs=1) as idp:
            ident_f = idp.tile([P, P], f32)
            make_identity(nc, ident_f[:])
            nc.vector.tensor_copy(ident[:], ident_f[:])
        eps_t = const_pool.tile([P, 1], f32)
        nc.vector.memset(eps_t[:], LN_EPS)
        ones2 = const_pool.tile([P, 2], f32)
        nc.vector.memset(ones2[:, 0:1], 1.0)
        nc.vector.memset(ones2[:, 1:2], 0.0)
        bf_t = const_pool.tile([P, FC], f32)
        nc.gpsimd.dma_start(bf_t[:], bf_r)
        # b2 as a rank-1 ones-row matmul operand (K=2: [1,0] rows x [b2; 0])
        onesr = const_pool.tile([2, P], f32r)
        b2row = const_pool.tile([2, D], f32r)
        with tc.tile_pool(name="b2f", bufs=1) as b2p:
            onesr_f = b2p.tile([2, P], f32)
            nc.vector.memset(onesr_f[:], 0.0)
            nc.vector.memset(onesr_f[0:1, :], 1.0)
            nc.vector.tensor_copy(onesr[:], onesr_f[:])
            b2row_f = b2p.tile([2, D], f32)
            nc.vector.memset(b2row_f[:], 0.0)
            nc.gpsimd.dma_start(b2row_f[0:1, :], b2_d[None, :])
            nc.vector.tensor_copy(b2row[:], b2row_f[:])
        aff = {}
        if has_affine1:
            aff[1] = (const_pool.tile([P, D], f32, tag="g1b"),
                      const_pool.tile([P, D], f32, tag="be1b"))
            nc.gpsimd.dma_start(aff[1][0][:], bcast(g1_d))
            nc.gpsimd.dma_start(aff[1][1][:], bcast(be1_d))
        if has_affine2:
            aff[2] = (const_pool.tile([P, D], f32, tag="g2b"),
                      const_pool.tile([P, D], f32, tag="be2b"))
            nc.gpsimd.dma_start(aff[2][0][:], bcast(g2_d))
            nc.gpsimd.dma_start(aff[2][1][:], bcast(be2_d))

        # LN2 batched stats (written in phase B, consumed in phase C)
        mv_all = const_pool.tile([P, SC, 2], f32)
        rstd_all = const_pool.tile([P, SC], f32)
        std_all = const_pool.tile([P, SC], f32)

        # ---- persistent activations / preloaded weights ------------------
        xb = long_pool.tile([P, SC, D], f32)             # x, becomes x2 in place
        hT = long_pool.tile([P, DC, S], f32r, tag="actT")  # hT, reused as h2T
        w1_t = w1_pool.tile([P, DC, F], f32r)

        def ln_apply(tmp_pool, i, mean, rstd, which):
            """(xb[:,i,:] - mean) * rstd [*gamma + beta] -> row-major tile."""
            h_t = tmp_pool.tile([P, D], f32r, tag="h_rm")
            nc.vector.tensor_scalar(out=h_t[:], in0=xb[:, i, :],
                                    scalar1=mean, scalar2=rstd,
                                    op0=OP.subtract, op1=OP.mult)
            if which in aff:
                g_b, be_b = aff[which]
                nc.vector.tensor_tensor(h_t[:], h_t[:], g_b[:], op=OP.mult)
                nc.vector.tensor_tensor(h_t[:], h_t[:], be_b[:], op=OP.add)
            return h_t

        def transpose_to(tr_psum, dest_T, i, h_t):
            for dj in range(DC):
                ps = tr_psum.tile([P, P], f32r, tag="tr")
                nc.tensor.transpose(ps[:], h_t[:, dj * P:(dj + 1) * P], ident[:])
                nc.vector.tensor_copy(dest_T[:, dj, i * P:(i + 1) * P], ps[:])

        # ================= phase A: LN1 + u/v projections =================
        with ExitStack() as ph:
            ph_qk = ph.enter_context(tc.tile_pool(name="uv", bufs=1))
            uT = ph_qk.tile([P, DC, S], f32r, tag="uT")
            v_aug = ph_qk.tile([P, SC, D + 2], f32r, tag="vaug")

            with ExitStack() as pha:
                wA_pool = pha.enter_context(tc.tile_pool(name="wA", bufs=1))
                tmpA = pha.enter_context(tc.tile_pool(name="tmpA", bufs=3))
                statsA = pha.enter_context(tc.tile_pool(name="statsA", bufs=4))
                tr_psA = pha.enter_context(tc.tile_pool(name="trpsA", bufs=2,
                                                        space="PSUM"))
                mm_psA = pha.enter_context(tc.tile_pool(name="mmpsA", bufs=5,
                                                        space="PSUM"))

                wu_t = wA_pool.tile([P, DC, D], f32r)
                nc.gpsimd.dma_start(wu_t[:], wu_r)
                wvo_t = wA_pool.tile([P, DC, D], f32r)
                nc.gpsimd.dma_start(wvo_t[:], wvo_r)
                # W1 preload early on the idle-after-that Pool queue
                nc.gpsimd.dma_start(w1_t[:], w1_r)

                for i in range(SC):
                    dma_eng = nc.sync if i % 2 == 0 else nc.scalar
                    dma_eng.dma_start(xb[:, i, :], x_r[:, i, :])
                    stats = statsA.tile([P, 6], f32, tag="bn_stats")
                    nc.vector.bn_stats(stats[:], xb[:, i, :])
                    mv = statsA.tile([P, 2], f32, tag="bn_aggr")
                    nc.vector.bn_aggr(mv[:], stats[:])
                    std = statsA.tile([P, 1], f32, tag="std")
                    nc.scalar.activation(std[:], mv[:, 1:2], AF.Sqrt,
                                         bias=eps_t[:], scale=1.0)
                    rstd = statsA.tile([P, 1], f32, tag="rstd")
                    nc.vector.reciprocal(rstd[:], std[:])
                    h_t = ln_apply(tmpA, i, mv[:, 0:1], rstd[:], 1)
                    transpose_to(tr_psA, hT, i, h_t)
                    # v' row-major for this t-chunk: [t, dout] = h @ Wvo
                    ps = mm_psA.tile([P, 512], f32, tag="proj")
                    for k in range(DC):
                        nc.tensor.matmul(ps[:], hT[:, k, i * P:(i + 1) * P],
                                         wvo_t[:, k, :],
                                         start=(k == 0), stop=(k == DC - 1))
                    nc.vector.tensor_copy(v_aug[:, i, 0:D], ps[:])
                    # uT s-tile as soon as its 4 h-chunks exist
                    if i % 4 == 3:
                        n = i // 4
                        for m in range(DC):
                            ps = mm_psA.tile([P, 512], f32, tag="proj")
                            for k in range(DC):
                                nc.tensor.matmul(
                                    ps[:], wu_t[:, k, m * P:(m + 1) * P],
                                    hT[:, k, n * 512:(n + 1) * 512],
                                    start=(k == 0), stop=(k == DC - 1))
                            nc.vector.tensor_copy(
                                uT[:, m, n * 512:(n + 1) * 512], ps[:])
                # ones/zero columns for the softmax denominator
                nc.vector.tensor_copy(
                    v_aug[:, :, D:D + 2],
                    ones2[:, None, :].to_broadcast((P, SC, 2)))

            # ============= phase B: attention (+ LN2 stats) ===============
            h2_pre = {}
            with ExitStack() as phb:
                pT_pool = phb.enter_context(tc.tile_pool(name="pT", bufs=2))
                sc_ps = phb.enter_context(tc.tile_pool(name="scps", bufs=4,
                                                       space="PSUM"))
                a_ps = phb.enter_context(tc.tile_pool(name="aps", bufs=2,
                                                      space="PSUM"))
                rec_pool = phb.enter_context(tc.tile_pool(name="rec", bufs=4))
                statsB = phb.enter_context(tc.tile_pool(name="statsB", bufs=4))

                pT_tiles = {}

                def attn_block(j):
                    pT = pT_tiles.pop(j)
                    for c in range(CPB):
                        scn = j * CPB + c
                        pa1 = a_ps.tile([P, 256], f32, tag="pa1")
                        pa2 = a_ps.tile([P, 258], f32, tag="pa2")
                        for m in range(SC):
                            nc.tensor.matmul(pa1[:],
                                             pT[:, m, c * P:(c + 1) * P],
                                             v_aug[:, m, 0:256],
                                             start=(m == 0), stop=(m == SC - 1))
                            nc.tensor.matmul(pa2[:],
                                             pT[:, m, c * P:(c + 1) * P],
                                             v_aug[:, m, 256:514],
                                             start=(m == 0), stop=(m == SC - 1))
                        rec = rec_pool.tile([P, 1], f32, tag="rec")
                        nc.vector.reciprocal(rec[:], pa2[:, 256:257])
                        nc.vector.scalar_tensor_tensor(
                            out=xb[:, scn, 0:256], in0=pa1[:], scalar=rec[:],
                            in1=xb[:, scn, 0:256], op0=OP.mult, op1=OP.add)
                        nc.vector.scalar_tensor_tensor(
                            out=xb[:, scn, 256:512], in0=pa2[:, 0:256],
                            scalar=rec[:], in1=xb[:, scn, 256:512],
                            op0=OP.mult, op1=OP.add)
                        # LN2 stats for this finished chunk (batched sqrt later)
                        stats = statsB.tile([P, 6], f32, tag="bn2")
                        nc.vector.bn_stats(stats[:], xb[:, scn, :])
                        nc.vector.bn_aggr(mv_all[:, scn, :], stats[:])

                def ln2_sqrt_group(g):
                    # one Sqrt+Reciprocal per 8 chunks: rstd ready before C
                    sl = slice(8 * g, 8 * (g + 1))
                    nc.scalar.activation(std_all[:, sl], mv_all[:, sl, 1],
                                         AF.Sqrt, bias=eps_t[:], scale=1.0)
                    nc.vector.reciprocal(rstd_all[:, sl], std_all[:, sl])

                for j in range(NB):
                    # scores^T for block j, two t-chunks per PSUM bank so one
                    # exp op covers a [128, 512] pair (ACT keeps PE pace)
                    pT = pT_pool.tile([P, SC, SB], f32r, tag="pT")
                    pT_tiles[j] = pT
                    for m in range(0, SC, 2):
                        ps = sc_ps.tile([P, 2 * SB], f32, tag="sc")
                        for half in range(2):
                            dst = ps[:, half * SB:(half + 1) * SB]
                            for k in range(DC):
                                nc.tensor.matmul(
                                    dst, uT[:, k, (m + half) * P:(m + half + 1) * P],
                                    hT[:, k, j * SB:(j + 1) * SB],
                                    start=(k == 0), stop=(k == DC - 1))
                        nc.scalar.activation(
                            pT[:, m:m + 2, :],
                            ps[:].rearrange("p (a b) -> p a b", a=2),
                            AF.Exp, scale=ATTN_SCALE)
                    # attention for the previous block (software pipelining)
                    if j > 0:
                        attn_block(j - 1)
                        if j - 1 == 3:
                            ln2_sqrt_group(0)
                            # LN2 applies for early chunks overlap phase B
                            for i in range(2):
                                h2_pre[i] = ln_apply(tmpBC, i, mv_all[:, i, 0:1],
                                                     rstd_all[:, i:i + 1], 2)
                attn_block(NB - 1)
                ln2_sqrt_group(1)

        # ================= phase C: LN2 apply + MLP =======================
        with ExitStack() as phc:
            wC_pool = phc.enter_context(tc.tile_pool(name="wC", bufs=1))
            gT_pool = phc.enter_context(tc.tile_pool(name="gT", bufs=2))
            tr_psC = phc.enter_context(tc.tile_pool(name="trpsC", bufs=2,
                                                    space="PSUM"))
            f1_ps = phc.enter_context(tc.tile_pool(name="f1ps", bufs=3,
                                                   space="PSUM"))
            y_ps = phc.enter_context(tc.tile_pool(name="yps", bufs=3,
                                                  space="PSUM"))

            # W2 split across the SP and Pool queues for fast arrival
            w2_t = wC_pool.tile([P, FC, D], f32r)
            for g in range(4):
                eng = nc.sync if g % 2 == 0 else nc.gpsimd
                eng.dma_start(w2_t[:, 4 * g:4 * (g + 1), :],
                              w2_r[:, 4 * g:4 * (g + 1), :])

            h2T = long_pool.tile([P, DC, S], f32r, tag="actT")  # reuses hT slot
            for i in range(SC):
                if i in h2_pre:
                    h_t = h2_pre.pop(i)
                else:
                    h_t = ln_apply(tmpBC, i, mv_all[:, i, 0:1],
                                   rstd_all[:, i:i + 1], 2)
                transpose_to(tr_psC, h2T, i, h_t)

            def fc2_block(jj, gT):
                for c in range(CPBM):
                    scn = jj * CPBM + c
                    ps = y_ps.tile([P, D], f32, tag="y")
                    for m in range(FC):
                        nc.tensor.matmul(ps[:], gT[:, m, c * P:(c + 1) * P],
                                         w2_t[:, m, :],
                                         start=(m == 0), stop=False)
                    nc.tensor.matmul(ps[:], onesr[:], b2row[:],
                                     start=False, stop=True)
                    nc.vector.tensor_tensor(xb[:, scn, :], ps[:],
                                            xb[:, scn, :], op=OP.add)
                    eng = nc.sync if scn % 2 == 0 else nc.gpsimd
                    eng.dma_start(out_r[:, scn, :], xb[:, scn, :])

            gT_tiles = {}
            for jj in range(NBM):
                gT = gT_pool.tile([P, FC, SBM], f32r, tag="gT")
                gT_tiles[jj] = gT
                for m in range(FC):
                    ps = f1_ps.tile([P, SBM], f32, tag="f1")
                    for k in range(DC):
                        nc.tensor.matmul(ps[:], w1_t[:, k, m * P:(m + 1) * P],
                                         h2T[:, k, jj * SBM:(jj + 1) * SBM],
                                         start=(k == 0), stop=(k == DC - 1))
                    nc.scalar.activation(gT[:, m, :], ps[:], AF.Gelu,
                                         bias=bf_t[:, m:m + 1], scale=1.0)
                # software pipelining: fc2 of the previous block
                if jj > 0:
                    fc2_block(jj - 1, gT_tiles.pop(jj - 1))
            fc2_block(NBM - 1, gT_tiles.pop(NBM - 1))

    nc.compile()
    return nc


def _fold_weights(inputs):
    """Host-side constant folding (float64): Wu = Wk Wq^T, Wvo = Wv Wo."""
    f64 = {k: np.asarray(v, dtype=np.float64) for k, v in inputs.items()}
    return {
        "wu": (f64["Wk"] @ f64["Wq"].T).astype(np.float32),
        "wvo": (f64["Wv"] @ f64["Wo"]).astype(np.float32),
        "w1": f64["W1"].astype(np.float32),
        "w2": f64["W2"].astype(np.float32),
        "bf": f64["b1"].astype(np.float32),
        "b2": f64["b2"].astype(np.float32),
        "g1": f64["g1"].astype(np.float32),
        "be1": f64["be1"].astype(np.float32),
        "g2": f64["g2"].astype(np.float32),
        "be2": f64["be2"].astype(np.float32),
    }


def _flags(inputs):
    has1 = not (np.all(np.asarray(inputs["g1"]) == 1.0)
                and np.all(np.asarray(inputs["be1"]) == 0.0))
    has2 = not (np.all(np.asarray(inputs["g2"]) == 1.0)
                and np.all(np.asarray(inputs["be2"]) == 0.0))
    return has1, has2


def _get_runner(flags):
    """Build (once per flag set) a cached jitted SPMD runner over 8 cores."""
    key = ("runner", flags)
    if key in _CACHE:
        return _CACHE[key]

    import jax
    import numpy as _np
    from jax.sharding import Mesh, PartitionSpec, NamedSharding
    from jax.experimental.shard_map import shard_map
    import concourse.mybir as mybir
    from concourse.bass2jax import (_bass_exec_p, install_neuronx_cc_hook,
                                    partition_id_tensor)

    nc = _build(*flags)
    install_neuronx_cc_hook()

    partition_name = (nc.partition_id_tensor.name
                      if nc.partition_id_tensor else None)
    in_names, out_names, out_avals, zero_outs = [], [], [], []
    for alloc in nc.m.functions[0].allocations:
        if not isinstance(alloc, mybir.MemoryLocationSet):
            continue
        name = alloc.memorylocations[0].name
        if alloc.kind == "ExternalInput":
            if name != partition_name:
                in_names.append(name)
        elif alloc.kind == "ExternalOutput":
            out_names.append(name)
            shape = tuple(alloc.tensor_shape)
            dtype = mybir.dt.np(alloc.dtype)
            out_avals.append(jax.core.ShapedArray(shape, dtype))
            zero_outs.append(_np.zeros(shape, dtype))
    n_params = len(in_names)
    all_in_names = in_names + out_names
    if partition_name is not None:
        all_in_names = all_in_names + [partition_name]

    def _body(*args):
        operands = list(args)
        if partition_name is not None:
            operands.append(partition_id_tensor())
        outs = _bass_exec_p.bind(
            *operands,
            out_avals=tuple(out_avals),
            in_names=tuple(all_in_names),
            out_names=tuple(out_names),
            lowering_input_output_aliases=(),
            sim_require_finite=True,
            sim_require_nnan=True,
            nc=nc,
        )
        return tuple(outs)

    devices = jax.devices()[:NCORES]
    mesh = Mesh(_np.asarray(devices), ("core",))
    n_all = n_params + len(out_names)
    sharded = jax.jit(
        shard_map(_body, mesh=mesh,
                  in_specs=(PartitionSpec("core"),) * n_all,
                  out_specs=(PartitionSpec("core"),) * len(out_names),
                  check_rep=False),
        keep_unused=True,
    )
    sharding = NamedSharding(mesh, PartitionSpec("core"))
    runner = {
        "sharded": sharded, "sharding": sharding, "in_names": in_names,
        "out_names": out_names, "zero_outs": zero_outs, "jax": jax,
        "np": _np,
    }
    _CACHE[key] = runner
    return runner


def _stage(inputs):
    """Shard + fold inputs, return staged device arrays for the runner."""
    flags = _flags(inputs)
    r = _get_runner(flags)
    jax, _np = r["jax"], r["np"]
    x = _np.asarray(inputs["x"], dtype=_np.float32)          # [8, 2048, 512]
    folded = _fold_weights(inputs)
    per_core = {"x": [x[c] for c in range(NCORES)]}
    for k, v in folded.items():
        per_core[k] = [v] * NCORES
    concat = []
    for name in r["in_names"]:
        concat.append(_np.concatenate([per_core[name][c] for c in range(NCORES)],
                                      axis=0))
    for z in r["zero_outs"]:
        concat.append(_np.zeros((NCORES * z.shape[0],) + z.shape[1:], z.dtype))
    return flags, [jax.device_put(a, r["sharding"]) for a in concat]


def _run_staged(flags, staged):
    r = _get_runner(flags)
    return r["sharded"](*staged)


def kernel(**inputs):
    flags, staged = _stage(inputs)
    outs = _run_staged(flags, staged)
    out = np.asarray(outs[0])                                # [8*2048, 512]
    return out.reshape(NCORES, S, D).astype(np.float32)


# revision 29
# speedup vs baseline: 1.1330x; 1.1330x over previous
"""MiniTransformer block on 8 Trainium2 NeuronCores.

Sharding: pure data-parallel over batch (B=8 -> 1 batch element per core,
no collectives). Per core the full transformer block (LN -> single-head
attention -> residual -> LN -> MLP -> residual) runs as one Bass/Tile kernel.

Key design points:
  * All matmuls run in float32r (TF32-like, 1 cycle/row on the PE at free
    dim >= 256 vs 4 cycles/row for fp32; measured fro rel err ~1.5e-4).
  * Activations for matmul consumption are kept transposed ([feature, token])
    so projections chain without transposes; only LN outputs are transposed
    (PE transpose, 4 per 128-row chunk).
  * Attention scores are computed via a host-folded Wu = Wk @ Wq^T:
    scores^T = (h Wu) . h, so only ONE projection (u) is materialized
    instead of q and k.
  * Softmax: scores are computed transposed [t, s]; exp (with the 1/sqrt(D)
    scale fused) happens on the ScalarE during PSUM eviction; no max
    subtraction (LN-bounded scores, fp32 exp range is ample); the
    denominator comes from an extra ones-column appended to v, landing in
    PSUM as a per-partition scalar; normalization + residual add fold into
    a single scalar_tensor_tensor eviction.
  * (p @ v) @ Wo is computed as p @ (v (Wv Wo)) via host-folded Wvo,
    removing a projection and a transpose.
  * DMA traffic is spread across the SP/ACT HWDGE queues and the Pool SWDGE
    queue so no engine's sequencer stalls compute.
"""

import numpy as np

S, D, F, P = 2048, 512, 2048, 128
SC, DC, FC = S // P, D // P, F // P  # 16, 4, 16
SB = 256                             # attention s-block
NB = S // SB                         # 8
CPB = SB // P                        # s-chunks per attention block = 2
SBM = 512                            # MLP s-block
NBM = S // SBM                       # 4
CPBM = SBM // P                      # s-chunks per MLP block = 4
NCORES = 8
LN_EPS = 1e-5
ATTN_SCALE = float(1.0 / np.sqrt(np.float32(D)))

_CACHE = {}


def _build(has_affine1, has_affine2):
    import concourse.bass as bass
    import concourse.mybir as mybir
    import concourse.tile as tile
    from concourse import bacc
    from concourse.masks import make_identity
    from contextlib import ExitStack

    f32 = mybir.dt.float32
    f32r = mybir.dt.float32r
    AF = mybir.ActivationFunctionType
    OP = mybir.AluOpType

    nc = bacc.Bacc("TRN2", target_bir_lowering=False, debug=False,
                   num_devices=NCORES)

    x_d = nc.dram_tensor("x", [S, D], f32, kind="ExternalInput").ap()
    wu_d = nc.dram_tensor("wu", [D, D], f32r, kind="ExternalInput").ap()
    wvo_d = nc.dram_tensor("wvo", [D, D], f32r, kind="ExternalInput").ap()
    w1_d = nc.dram_tensor("w1", [D, F], f32r, kind="ExternalInput").ap()
    w2_d = nc.dram_tensor("w2", [F, D], f32r, kind="ExternalInput").ap()
    bf_d = nc.dram_tensor("bf", [F], f32, kind="ExternalInput").ap()
    b2_d = nc.dram_tensor("b2", [D], f32, kind="ExternalInput").ap()
    g1_d = nc.dram_tensor("g1", [D], f32, kind="ExternalInput").ap()
    be1_d = nc.dram_tensor("be1", [D], f32, kind="ExternalInput").ap()
    g2_d = nc.dram_tensor("g2", [D], f32, kind="ExternalInput").ap()
    be2_d = nc.dram_tensor("be2", [D], f32, kind="ExternalInput").ap()
    out_d = nc.dram_tensor("out", [S, D], f32, kind="ExternalOutput").ap()

    x_r = x_d.rearrange("(sc p) d -> p sc d", p=P)      # [128, 16, 512]
    out_r = out_d.rearrange("(sc p) d -> p sc d", p=P)
    wu_r = wu_d.rearrange("(ko ki) n -> ki ko n", ki=P)  # [128, 4, 512]
    wvo_r = wvo_d.rearrange("(ko ki) n -> ki ko n", ki=P)
    w1_r = w1_d.rearrange("(ko ki) n -> ki ko n", ki=P)  # [128, 4, 2048]
    w2_r = w2_d.rearrange("(ko ki) n -> ki ko n", ki=P)  # [128, 16, 512]
    bf_r = bf_d.rearrange("(o p) -> p o", p=P)           # [128, 16]

    def bcast(ap):  # [D] -> [128, D] partition-broadcast (for DMA only)
        return bass.AP(tensor=ap.tensor, offset=ap.offset, ap=[[0, P], [1, D]])

    with tile.TileContext(nc) as tc, ExitStack() as top:
        long_pool = top.enter_context(tc.tile_pool(name="long", bufs=1))
        const_pool = top.enter_context(tc.tile_pool(name="consts", bufs=1))
        w1_pool = top.enter_context(tc.tile_pool(name="w1p", bufs=1))
        tmpBC = top.enter_context(tc.tile_pool(name="tmpBC", bufs=2))

        # ---- constants / small tiles -------------------------------------
        ident = const_pool.tile([P, P], f32r)
        with tc.tile_pool(name="identf", bufs=1) as idp:
            ident_f = idp.tile([P, P], f32)
            make_identity(nc, ident_f[:])
            nc.vector.tensor_copy(ident[:], ident_f[:])
        eps_t = const_pool.tile([P, 1], f32)
        nc.vector.memset(eps_t[:], LN_EPS)
        ones2 = const_pool.tile([P, 2], f32)
        nc.vector.memset(ones2[:, 0:1], 1.0)
        nc.vector.memset(ones2[:, 1:2], 0.0)
        bf_t = const_pool.tile([P, FC], f32)
        nc.gpsimd.dma_start(bf_t[:], bf_r)
        # b2 as a rank-1 ones-row matmul operand (K=2: [1,0] rows x [b2; 0])
        onesr = const_pool.tile([2, P], f32r)
        b2row = const_pool.tile([2, D], f32r)
        with tc.tile_pool(name="b2f", bufs=1) as b2p:
            onesr_f = b2p.tile([2, P], f32)
            nc.vector.memset(onesr_f[:], 0.0)
            nc.vector.memset(onesr_f[0:1, :], 1.0)
            nc.vector.tensor_copy(onesr[:], onesr_f[:])
            b2row_f = b2p.tile([2, D], f32)
            nc.vector.memset(b2row_f[:], 0.0)
            nc.gpsimd.dma_start(b2row_f[0:1, :], b2_d[None, :])
            nc.vector.tensor_copy(b2row[:], b2row_f[:])
        aff = {}
        if has_affine1:
            aff[1] = (const_pool.tile([P, D], f32, tag="g1b"),
                      const_pool.tile([P, D], f32, tag="be1b"))
            nc.gpsimd.dma_start(aff[1][0][:], bcast(g1_d))
            nc.gpsimd.dma_start(aff[1][1][:], bcast(be1_d))
        if has_affine2:
            aff[2] = (const_pool.tile([P, D], f32, tag="g2b"),
                      const_pool.tile([P, D], f32, tag="be2b"))
            nc.gpsimd.dma_start(aff[2][0][:], bcast(g2_d))
            nc.gpsimd.dma_start(aff[2][1][:], bcast(be2_d))

        # LN2 batched stats (written in phase B, consumed in phase C)
        mv_all = const_pool.tile([P, SC, 2], f32)
        rstd_all = const_pool.tile([P, SC], f32)
        std_all = const_pool.tile([P, SC], f32)

        # ---- persistent activations / preloaded weights ------------------
        xb = long_pool.tile([P, SC, D], f32)             # x, becomes x2 in place
        hT = long_pool.tile([P, DC, S], f32r, tag="actT")  # hT, reused as h2T
        w1_t = w1_pool.tile([P, DC, F], f32r)

        def ln_apply(tmp_pool, i, mean, rstd, which):
            """(xb[:,i,:] - mean) * rstd [*gamma + beta] -> row-major tile."""
            h_t = tmp_pool.tile([P, D], f32r, tag="h_rm")
            nc.vector.tensor_scalar(out=h_t[:], in0=xb[:, i, :],
                                    scalar1=mean, scalar2=rstd,
                                    op0=OP.subtract, op1=OP.mult)
            if which in aff:
                g_b, be_b = aff[which]
                nc.vector.tensor_tensor(h_t[:], h_t[:], g_b[:], op=OP.mult)
                nc.vector.tensor_tensor(h_t[:], h_t[:], be_b[:], op=OP.add)
            return h_t

        def transpose_to(tr_psum, dest_T, i, h_t):
            for dj in range(DC):
                ps = tr_psum.tile([P, P], f32r, tag="tr")
                nc.tensor.transpose(ps[:], h_t[:, dj * P:(dj + 1) * P], ident[:])
                nc.vector.tensor_copy(dest_T[:, dj, i * P:(i + 1) * P], ps[:])

        # ================= phase A: LN1 + u/v projections =================
        with ExitStack() as ph:
            ph_qk = ph.enter_context(tc.tile_pool(name="uv", bufs=1))
            uT = ph_qk.tile([P, DC, S], f32r, tag="uT")
            v_aug = ph_qk.tile([P, SC, D + 2], f32r, tag="vaug")

            with ExitStack() as pha:
                wA_pool = pha.enter_context(tc.tile_pool(name="wA", bufs=1))
                tmpA = pha.enter_context(tc.tile_pool(name="tmpA", bufs=3))
                statsA = pha.enter_context(tc.tile_pool(name="statsA", bufs=4))
                tr_psA = pha.enter_context(tc.tile_pool(name="trpsA", bufs=2,
                                                        space="PSUM"))
                mm_psA = pha.enter_context(tc.tile_pool(name="mmpsA", bufs=5,
                                                        space="PSUM"))

                wu_t = wA_pool.tile([P, DC, D], f32r)
                nc.gpsimd.dma_start(wu_t[:], wu_r)
                wvo_t = wA_pool.tile([P, DC, D], f32r)
                nc.gpsimd.dma_start(wvo_t[:], wvo_r)
                # W1 preload early on the idle-after-that Pool queue
                nc.gpsimd.dma_start(w1_t[:], w1_r)

                for i in range(SC):
                    dma_eng = nc.sync if i % 2 == 0 else nc.scalar
                    dma_eng.dma_start(xb[:, i, :], x_r[:, i, :])
                    stats = statsA.tile([P, 6], f32, tag="bn_stats")
                    nc.vector.bn_stats(stats[:], xb[:, i, :])
                    mv = statsA.tile([P, 2], f32, tag="bn_aggr")
                    nc.vector.bn_aggr(mv[:], stats[:])
                    std = statsA.tile([P, 1], f32, tag="std")
                    nc.scalar.activation(std[:], mv[:, 1:2], AF.Sqrt,
                                         bias=eps_t[:], scale=1.0)
                    rstd = statsA.tile([P, 1], f32, tag="rstd")
                    nc.vector.reciprocal(rstd[:], std[:])
                    h_t = ln_apply(tmpA, i, mv[:, 0:1], rstd[:], 1)
                    transpose_to(tr_psA, hT, i, h_t)
                    # v' row-major for this t-chunk: [t, dout] = h @ Wvo
                    ps = mm_psA.tile([P, 512], f32, tag="proj")
                    for k in range(DC):
                        nc.tensor.matmul(ps[:], hT[:, k, i * P:(i + 1) * P],
                                         wvo_t[:, k, :],
                                         start=(k == 0), stop=(k == DC - 1))
                    nc.vector.tensor_copy(v_aug[:, i, 0:D], ps[:])
                    # uT s-tile as soon as its 4 h-chunks exist
                    if i % 4 == 3:
                        n = i // 4
                        for m in range(DC):
                            ps = mm_psA.tile([P, 512], f32, tag="proj")
                            for k in range(DC):
                                nc.tensor.matmul(
                                    ps[:], wu_t[:, k, m * P:(m + 1) * P],
                                    hT[:, k, n * 512:(n + 1) * 512],
                                    start=(k == 0), stop=(k == DC - 1))
                            nc.vector.tensor_copy(
                                uT[:, m, n * 512:(n + 1) * 512], ps[:])
                # ones/zero columns for the softmax denominator
                nc.vector.tensor_copy(
                    v_aug[:, :, D:D + 2],
                    ones2[:, None, :].to_broadcast((P, SC, 2)))

            # ============= phase B: attention (+ LN2 stats) ===============
            h2_pre = {}
            with ExitStack() as phb:
                pT_pool = phb.enter_context(tc.tile_pool(name="pT", bufs=2))
                sc_ps = phb.enter_context(tc.tile_pool(name="scps", bufs=4,
                                                       space="PSUM"))
                a_ps = phb.enter_context(tc.tile_pool(name="aps", bufs=2,
                                                      space="PSUM"))
                rec_pool = phb.enter_context(tc.tile_pool(name="rec", bufs=4))
                statsB = phb.enter_context(tc.tile_pool(name="statsB", bufs=4))

                pT_tiles = {}

                def attn_block(j):
                    pT = pT_tiles.pop(j)
                    for c in range(CPB):
                        scn = j * CPB + c
                        pa1 = a_ps.tile([P, 256], f32, tag="pa1")
                        pa2 = a_ps.tile([P, 258], f32, tag="pa2")
                        for m in range(SC):
                            nc.tensor.matmul(pa1[:],
                                             pT[:, m, c * P:(c + 1) * P],
                                             v_aug[:, m, 0:256],
                                             start=(m == 0), stop=(m == SC - 1))
                            nc.tensor.matmul(pa2[:],
                                             pT[:, m, c * P:(c + 1) * P],
                                             v_aug[:, m, 256:514],
                                             start=(m == 0), stop=(m == SC - 1))
                        rec = rec_pool.tile([P, 1], f32, tag="rec")
                        nc.vector.reciprocal(rec[:], pa2[:, 256:257])
                        nc.vector.scalar_tensor_tensor(
                            out=xb[:, scn, 0:256], in0=pa1[:], scalar=rec[:],
                            in1=xb[:, scn, 0:256], op0=OP.mult, op1=OP.add)
                        nc.vector.scalar_tensor_tensor(
                            out=xb[:, scn, 256:512], in0=pa2[:, 0:256],
                            scalar=rec[:], in1=xb[:, scn, 256:512],
                            op0=OP.mult, op1=OP.add)
                        # LN2 stats for this finished chunk (batched sqrt later)
                        stats = statsB.tile([P, 6], f32, tag="bn2")
                        nc.vector.bn_stats(stats[:], xb[:, scn, :])
                        nc.vector.bn_aggr(mv_all[:, scn, :], stats[:])

                def ln2_sqrt_group(g):
                    # one Sqrt+Reciprocal per 8 chunks: rstd ready before C
                    sl = slice(8 * g, 8 * (g + 1))
                    nc.scalar.activation(std_all[:, sl], mv_all[:, sl, 1],
                                         AF.Sqrt, bias=eps_t[:], scale=1.0)
                    nc.vector.reciprocal(rstd_all[:, sl], std_all[:, sl])

                for j in range(NB):
                    # scores^T for block j, two t-chunks per PSUM bank so one
                    # exp op covers a [128, 512] pair (ACT keeps PE pace)
                    pT = pT_pool.tile([P, SC, SB], f32r, tag="pT")
                    pT_tiles[j] = pT
                    for m in range(0, SC, 2):
                        ps = sc_ps.tile([P, 2 * SB], f32, tag="sc")
                        for half in range(2):
                            dst = ps[:, half * SB:(half + 1) * SB]
                            for k in range(DC):
                                nc.tensor.matmul(
                                    dst, uT[:, k, (m + half) * P:(m + half + 1) * P],
                                    hT[:, k, j * SB:(j + 1) * SB],
                                    start=(k == 0), stop=(k == DC - 1))
                        nc.scalar.activation(
                            pT[:, m:m + 2, :],
                            ps[:].rearrange("p (a b) -> p a b", a=2),
                            AF.Exp, scale=ATTN_SCALE)
                    # attention for the previous block (software pipelining)
                    if j > 0:
                        attn_block(j - 1)
                        if j - 1 == 3:
                            ln2_sqrt_group(0)
                            # LN2 applies for early chunks overlap phase B
                            for i in range(2):
                                h2_pre[i] = ln_apply(tmpBC, i, mv_all[:, i, 0:1],
                                                     rstd_all[:, i:i + 1], 2)
                attn_block(NB - 1)
                ln2_sqrt_group(1)

        # ================= phase C: LN2 apply + MLP =======================
        with ExitStack() as phc:
            wC_pool = phc.enter_context(tc.tile_pool(name="wC", bufs=1))
            gT_pool = phc.enter_context(tc.tile_pool(name="gT", bufs=2))
            tr_psC = phc.enter_context(tc.tile_pool(name="trpsC", bufs=2,
                                                    space="PSUM"))
            f1_ps = phc.enter_context(tc.tile_pool(name="f1ps", bufs=3,
                                                   space="PSUM"))
            y_ps = phc.enter_context(tc.tile_pool(name="yps", bufs=3,
                                                  space="PSUM"))

            # W2 split across the SP and Pool queues for fast arrival
            w2_t = wC_pool.tile([P, FC, D], f32r)
            for g in range(4):
                eng = nc.sync if g % 2 == 0 else nc.gpsimd
                eng.dma_start(w2_t[:, 4 * g:4 * (g + 1), :],
                              w2_r[:, 4 * g:4 * (g + 1), :])

            h2T = long_pool.tile([P, DC, S], f32r, tag="actT")  # reuses hT slot
            for i in range(SC):
                if i in h2_pre:
                    h_t = h2_pre.pop(i)
                else:
                    h_t = ln_apply(tmpBC, i, mv_all[:, i, 0:1],
                                   rstd_all[:, i:i + 1], 2)
                transpose_to(tr_psC, h2T, i, h_t)

            def fc2_chunk(jj, gT, c):
                scn = jj * CPBM + c
                ps = y_ps.tile([P, D], f32, tag="y")
                for m in range(FC):
                    nc.tensor.matmul(ps[:], gT[:, m, c * P:(c + 1) * P],
                                     w2_t[:, m, :],
                                     start=(m == 0), stop=False)
                nc.tensor.matmul(ps[:], onesr[:], b2row[:],
                                 start=False, stop=True)
                nc.vector.tensor_tensor(xb[:, scn, :], ps[:],
                                        xb[:, scn, :], op=OP.add)
                eng = nc.sync if scn % 2 == 0 else nc.gpsimd
                eng.dma_start(out_r[:, scn, :], xb[:, scn, :])

            gT_tiles = {}
            for jj in range(NBM):
                gT = gT_pool.tile([P, FC, SBM], f32r, tag="gT")
                gT_tiles[jj] = gT
                for m in range(FC):
                    ps = f1_ps.tile([P, SBM], f32, tag="f1")
                    for k in range(DC):
                        nc.tensor.matmul(ps[:], w1_t[:, k, m * P:(m + 1) * P],
                                         h2T[:, k, jj * SBM:(jj + 1) * SBM],
                                         start=(k == 0), stop=(k == DC - 1))
                    nc.scalar.activation(gT[:, m, :], ps[:], AF.Gelu,
                                         bias=bf_t[:, m:m + 1], scale=1.0)
                    # software pipelining: interleave previous block's fc2
                    if jj > 0 and m % 4 == 3:
                        fc2_chunk(jj - 1, gT_tiles[jj - 1], m // 4)
                if jj > 0:
                    gT_tiles.pop(jj - 1)
            for c in range(CPBM):
                fc2_chunk(NBM - 1, gT_tiles[NBM - 1], c)

    nc.compile()
    return nc


def _fold_weights(inputs):
    """Host-side constant folding (float64): Wu = Wk Wq^T, Wvo = Wv Wo."""
    f64 = {k: np.asarray(v, dtype=np.float64) for k, v in inputs.items()}
    return {
        "wu": (f64["Wk"] @ f64["Wq"].T).astype(np.float32),
        "wvo": (f64["Wv"] @ f64["Wo"]).astype(np.float32),
        "w1": f64["W1"].astype(np.float32),
        "w2": f64["W2"].astype(np.float32),
        "bf": f64["b1"].astype(np.float32),
        "b2": f64["b2"].astype(np.float32),
        "g1": f64["g1"].astype(np.float32),
        "be1": f64["be1"].astype(np.float32),
        "g2": f64["g2"].astype(np.float32),
        "be2": f64["be2"].astype(np.float32),
    }


def _flags(inputs):
    has1 = not (np.all(np.asarray(inputs["g1"]) == 1.0)
                and np.all(np.asarray(inputs["be1"]) == 0.0))
    has2 = not (np.all(np.asarray(inputs["g2"]) == 1.0)
                and np.all(np.asarray(inputs["be2"]) == 0.0))
    return has1, has2


def _get_runner(flags):
    """Build (once per flag set) a cached jitted SPMD runner over 8 cores."""
    key = ("runner", flags)
    if key in _CACHE:
        return _CACHE[key]

    import jax
    import numpy as _np
    from jax.sharding import Mesh, PartitionSpec, NamedSharding
    from jax.experimental.shard_map import shard_map
    import concourse.mybir as mybir
    from concourse.bass2jax import (_bass_exec_p, install_neuronx_cc_hook,
                                    partition_id_tensor)

    nc = _build(*flags)
    install_neuronx_cc_hook()

    partition_name = (nc.partition_id_tensor.name
                      if nc.partition_id_tensor else None)
    in_names, out_names, out_avals, zero_outs = [], [], [], []
    for alloc in nc.m.functions[0].allocations:
        if not isinstance(alloc, mybir.MemoryLocationSet):
            continue
        name = alloc.memorylocations[0].name
        if alloc.kind == "ExternalInput":
            if name != partition_name:
                in_names.append(name)
        elif alloc.kind == "ExternalOutput":
            out_names.append(name)
            shape = tuple(alloc.tensor_shape)
            dtype = mybir.dt.np(alloc.dtype)
            out_avals.append(jax.core.ShapedArray(shape, dtype))
            zero_outs.append(_np.zeros(shape, dtype))
    n_params = len(in_names)
    all_in_names = in_names + out_names
    if partition_name is not None:
        all_in_names = all_in_names + [partition_name]

    def _body(*args):
        operands = list(args)
        if partition_name is not None:
            operands.append(partition_id_tensor())
        outs = _bass_exec_p.bind(
            *operands,
            out_avals=tuple(out_avals),
            in_names=tuple(all_in_names),
            out_names=tuple(out_names),
            lowering_input_output_aliases=(),
            sim_require_finite=True,
            sim_require_nnan=True,
            nc=nc,
        )
        return tuple(outs)

    devices = jax.devices()[:NCORES]
    mesh = Mesh(_np.asarray(devices), ("core",))
    n_all = n_params + len(out_names)
    sharded = jax.jit(
        shard_map(_body, mesh=mesh,
                  in_specs=(PartitionSpec("core"),) * n_all,
                  out_specs=(PartitionSpec("core"),) * len(out_names),
                  check_rep=False),
        keep_unused=True,
    )
    sharding = NamedSharding(mesh, PartitionSpec("core"))
    runner = {
        "sharded": sharded, "sharding": sharding, "in_names": in_names,
        "out_names": out_names, "zero_outs": zero_outs, "jax": jax,
        "np": _np,
    }
    _CACHE[key] = runner
    return runner


def _stage(inputs):
    """Shard + fold inputs, return staged device arrays for the runner."""
    flags = _flags(inputs)
    r = _get_runner(flags)
    jax, _np = r["jax"], r["np"]
    x = _np.asarray(inputs["x"], dtype=_np.float32)          # [8, 2048, 512]
    folded = _fold_weights(inputs)
    per_core = {"x": [x[c] for c in range(NCORES)]}
    for k, v in folded.items():
        per_core[k] = [v] * NCORES
    concat = []
    for name in r["in_names"]:
        concat.append(_np.concatenate([per_core[name][c] for c in range(NCORES)],
                                      axis=0))
    for z in r["zero_outs"]:
        concat.append(_np.zeros((NCORES * z.shape[0],) + z.shape[1:], z.dtype))
    return flags, [jax.device_put(a, r["sharding"]) for a in concat]


def _run_staged(flags, staged):
    r = _get_runner(flags)
    return r["sharded"](*staged)


def kernel(**inputs):
    flags, staged = _stage(inputs)
    outs = _run_staged(flags, staged)
    out = np.asarray(outs[0])                                # [8*2048, 512]
    return out.reshape(NCORES, S, D).astype(np.float32)
